# revision 39
# baseline (speedup 1.0000x reference)
"""BPMLL loss kernel for Trainium2, data-parallel over 8 NeuronCores.

Reference computation (per sample row i of c [B, L], y [B, L] in {0,1}):
    pos_i  = sum_l y_il * exp(-c_il)
    neg_i  = sum_l (1 - y_il) * exp(c_il)
    Sy_i   = sum_l y_il
    loss_i = pos_i * neg_i / (Sy_i * (L - Sy_i))
    out    = mean_i loss_i                      (scalar, float32)

Device strategy: shard the batch dim across 8 cores (2048 rows each). The
label masking is folded into the exponent: with s = M*y - c and M = 128,
    exp(-s)     = exp(c - M*y)     -> (1-y)*exp(c)   (y=1 underflows to 0)
    exp(s - M)  = exp(-c + M*(y-1))-> y*exp(-c)      (y=0 underflows to 0)
so ScalarE's fused activation-with-accumulate computes each masked row sum
in a single pass.

The host packs each [128, 1024] row-tile pair into one contiguous block:
per partition row, 4096 B of c (f32) followed by 1024 B of y (int8 - the
mask is 0/1 so the downcast is lossless and cuts DMA bytes by 37%). Each
tile arrives in a single 640 KB SWDGE DMA; the kernel bitcasts the two
regions back to f32 / int8 on-chip. Per tile the device does: one DVE
scalar_tensor_tensor (s = y*M - c), one DVE reduce_sum over y, and two
ScalarE exp+accum passes. Each core emits [128, 48] row statistics
(pos, neg, Sy); the host finishes the tiny per-row division and the
global mean in float64.
"""

import numpy as np

B, L = 16384, 1024
N_CORES = 8
BS = B // N_CORES  # 2048 rows per core
P = 128
NSEG = BS // P  # 16 tiles of [128, L] per core
MASK = 128.0
ROWB = 4 * L + L  # bytes per partition row: c (f32) + y (int8)
DGE = "gpsimd"  # which engine issues the input loads: "gpsimd" or "sync"
IO_BUFS = 4


def _build_nc():
    import concourse.bacc as bacc
    import concourse.mybir as mybir
    from concourse.tile import TileContext

    f32 = mybir.dt.float32
    i8 = mybir.dt.int8
    u8 = mybir.dt.uint8

    # Skip the Bass-init all-engine barrier (~2-3 us): it only orders the
    # const-AP memsets, which this kernel never reads (bias APs are passed
    # explicitly below), and TileContext emits its own entry barrier.
    _orig_barrier = bacc.Bacc.all_engine_barrier
    bacc.Bacc.all_engine_barrier = lambda self: None
    try:
        nc = bacc.Bacc()
    finally:
        bacc.Bacc.all_engine_barrier = _orig_barrier
    cy_in = nc.dram_tensor("cy", [NSEG, P, ROWB], u8, kind="ExternalInput")
    stats = nc.dram_tensor("stats", [P, 3 * NSEG], f32, kind="ExternalOutput")

    with TileContext(nc) as tc:
        with (
            tc.tile_pool(name="io", bufs=IO_BUFS) as io,
            tc.tile_pool(name="psum_s", bufs=3, space="PSUM") as spool,
            tc.tile_pool(name="scratch", bufs=1) as scratch,
            tc.tile_pool(name="accs", bufs=1) as accs,
        ):
            allst = accs.tile([P, 3 * NSEG], f32)
            pos = allst[:, 0:NSEG]
            neg = allst[:, NSEG : 2 * NSEG]
            ysum = allst[:, 2 * NSEG : 3 * NSEG]
            neg_mask = accs.tile([P, 1], f32)
            nc.vector.memset(neg_mask[:], -MASK)
            zero_bias = accs.tile([P, 1], f32)
            nc.vector.memset(zero_bias[:], 0.0)
            # Each exp dumps its (unused) elementwise output into a private
            # region: disjoint ranges carry no WAW deps, so the exp
            # instructions need no event-semaphores between them.
            scrA = scratch.tile([P, NSEG * L], f32)
            scrB = scratch.tile([P, NSEG * L], f32)

            dma_eng = nc.gpsimd if DGE == "gpsimd" else nc.sync
            # The Sy reduce is pipelined one iteration behind the stt so
            # every stt (ScalarE's input) issues as early as possible; the
            # reduce fills DVE's idle slot while ScalarE consumes s.
            prev_reduce = None
            for i in range(NSEG):
                t = io.tile([P, ROWB], u8, tag="cy")
                # Segments 0 and 1 ride the two HWDGE queues (nc.sync /
                # nc.scalar) while the rest use the gpsimd SWDGE queue: DMA
                # bandwidth is shared round-robin per queue row, so the first
                # two tiles ride dedicated rows and land well before tiles
                # queued behind the SWDGE backlog - the exp stream starts
                # earlier and its first handoffs have slack.
                if i == 0:
                    eng = nc.sync
                elif i == 1:
                    eng = nc.scalar
                else:
                    eng = dma_eng
                eng.dma_start(t[:], cy_in[i])
                c_ap = t[:, 0 : 4 * L].bitcast(f32)
                y_ap = t[:, 4 * L : ROWB].bitcast(i8)

                s = spool.tile([P, L], f32, tag="s")
                nc.vector.scalar_tensor_tensor(
                    s[:],
                    y_ap,
                    MASK,
                    c_ap,
                    mybir.AluOpType.mult,
                    mybir.AluOpType.subtract,
                )
                if prev_reduce is not None:
                    py, pi = prev_reduce
                    nc.vector.reduce_sum(
                        ysum[:, pi : pi + 1], py, axis=mybir.AxisListType.X
                    )
                prev_reduce = (y_ap, i)
                nc.scalar.activation(
                    scrA[:, i * L : (i + 1) * L],
                    s[:],
                    mybir.ActivationFunctionType.Exp,
                    bias=zero_bias[:],
                    scale=-1.0,
                    accum_out=neg[:, i : i + 1],
                )
                nc.scalar.activation(
                    scrB[:, i * L : (i + 1) * L],
                    s[:],
                    mybir.ActivationFunctionType.Exp,
                    bias=neg_mask[:],
                    scale=1.0,
                    accum_out=pos[:, i : i + 1],
                )

            py, pi = prev_reduce
            nc.vector.reduce_sum(
                ysum[:, pi : pi + 1], py, axis=mybir.AxisListType.X
            )

            nc.sync.dma_start(stats[:], allst[:])

    nc.finalize()
    return nc


def _run(nc, in_maps, **kwargs):
    from concourse.bass_utils import run_bass_kernel_spmd

    return run_bass_kernel_spmd(nc, in_maps, list(range(N_CORES)), **kwargs)


def kernel(c, y, _bench_kwargs=None, _bench_result=None):
    c = np.asarray(c, dtype=np.float32)
    y = np.asarray(y, dtype=np.int32)
    assert c.shape == (B, L) and y.shape == (B, L)

    # Pack per [128, L] row-tile: per partition row 4096 B of c then 1024 B
    # of y as int8, so each tile is one contiguous 640 KB DMA.
    cyv = np.empty((N_CORES, NSEG, P, ROWB), np.uint8)
    cb = np.ascontiguousarray(c).view(np.uint8).reshape(N_CORES, NSEG, P, 4 * L)
    cyv[..., : 4 * L] = cb
    cyv[..., 4 * L :] = y.astype(np.uint8).reshape(N_CORES, NSEG, P, L)

    nc = _build_nc()
    in_maps = [{"cy": cyv[k]} for k in range(N_CORES)]
    res = _run(nc, in_maps, **(_bench_kwargs or {}))
    if _bench_result is not None:
        _bench_result.append(res)

    stats = np.stack([r["stats"] for r in res.results])  # [8, 128, 48]
    pos = stats[:, :, 0:NSEG].astype(np.float64)
    neg = stats[:, :, NSEG : 2 * NSEG].astype(np.float64)
    sy = stats[:, :, 2 * NSEG : 3 * NSEG].astype(np.float64)
    loss = pos * neg / (sy * (L - sy))
    return np.asarray(loss.mean(), dtype=np.float32)
